# revision 4
# baseline (speedup 1.0000x reference)
"""Trainium2 Bass kernel for nn_ConfidenceLossV2 (segment_reduce).

Strategy: pure data parallel over the batch dim (B=8 -> 8 NeuronCores,
one batch element per core). Each core computes:
  - seg_stats [64, 3]: per-segment (sum of channel-SUM sq err, pos count,
    pixel count) over the 128x128 subsampled grid
  - recov_stats [128, 4]: per-partition partial sums for the recovery loss
    (sum pos*d^2 for channels 0..2, sum pos)
Host gathers the tiny per-core partials and finishes the scalar math
(psum of (weighted sum, count) pairs), matching the reference exactly.
"""
import sys

if "/opt/trn_rl_repo" not in sys.path:
    sys.path.insert(0, "/opt/trn_rl_repo")

import numpy as np

B, C, H, W = 8, 3, 512, 512
CF, HF, WF = 64, 128, 128
G = 64
P = 128
WALL_COT = 0.5
NPIX = float(HF * WF)

_CACHE = {}


def _build():
    import concourse.bass as bass  # noqa: F401
    import concourse.tile as tile
    from concourse import bacc, mybir

    f32, i32 = mybir.dt.float32, mybir.dt.int32
    Alu = mybir.AluOpType
    Act = mybir.ActivationFunctionType

    nc = bacc.Bacc("TRN2", target_bir_lowering=False, debug=False, num_devices=B)

    t_out = nc.declare_dram_parameter("outputs", [C, H, W], f32, isOutput=False)
    t_in = nc.declare_dram_parameter("inputs", [C, H, W], f32, isOutput=False)
    t_enc = nc.declare_dram_parameter("enc1", [CF, HF, WF], f32, isOutput=False)
    t_dec = nc.declare_dram_parameter("dec1", [CF, HF, WF], f32, isOutput=False)
    t_mask = nc.declare_dram_parameter("masks", [H, W], f32, isOutput=False)
    t_seg = nc.declare_dram_parameter("segs", [H, W], i32, isOutput=False)
    t_iota = nc.declare_dram_parameter("iota", [P, G], f32, isOutput=False)
    t_segstats = nc.declare_dram_parameter("seg_stats", [G, 3], f32, isOutput=True)
    t_recov = nc.declare_dram_parameter("recov_stats", [P, 4], f32, isOutput=True)

    FW = 2048  # free width of a [512,512] image tiled as [128, 2048]
    EF = CF * WF  # 8192 free for enc/dec as [128(h), 64(c)*128(w)]

    with tile.TileContext(nc) as tc:
        with (
            tc.tile_pool(name="persist", bufs=1) as pp,
            tc.tile_pool(name="img", bufs=2) as ip,
            tc.tile_pool(name="scr", bufs=1) as sp,
            tc.tile_pool(name="big", bufs=1) as bp,
            tc.tile_pool(name="small", bufs=1) as mp,
            tc.tile_pool(name="psum", bufs=1, space="PSUM") as qp,
        ):
            # ---------------- constants / accumulators ----------------
            IO = pp.tile([P, G], f32, tag="iota")
            nc.sync.dma_start(out=IO[:], in_=t_iota[:])
            racc = pp.tile([P, 4], f32, tag="racc")

            # ---------------- enc/dec reconstruction error -------------
            # layout [128 (h), 64 (c) * 128 (w)]
            E = bp.tile([P, EF], f32, tag="bigA")
            nc.sync.dma_start(
                out=E[:].rearrange("p (c w) -> p c w", c=CF),
                in_=t_enc[:].rearrange("c h w -> h c w"),
            )
            D = bp.tile([P, EF], f32, tag="bigB")
            nc.sync.dma_start(
                out=D[:].rearrange("p (c w) -> p c w", c=CF),
                in_=t_dec[:].rearrange("c h w -> h c w"),
            )
            DF = bp.tile([P, EF], f32, tag="bigC")
            nc.vector.tensor_sub(DF[:], E[:], D[:])
            # square in place (ACT engine)
            nc.scalar.activation(out=DF[:], in_=DF[:], func=Act.Square)
            # err[h, w] = sum_c sq[h, (c, w)]
            err_t = mp.tile([P, WF], f32, tag="err")
            nc.vector.reduce_sum(
                out=err_t[:],
                in_=DF[:].rearrange("p (c w) -> p w c", c=CF),
                axis=mybir.AxisListType.X,
            )

            # ---------------- masks (full res, shared) -----------------
            M = pp.tile([P, FW], f32, tag="M")
            nc.sync.dma_start(
                out=M[:], in_=t_mask[:].rearrange("(p r) w -> p (r w)", p=P)
            )
            m01 = pp.tile([P, FW], f32, tag="m01")
            nc.vector.tensor_scalar(
                out=m01[:], in0=M[:], scalar1=WALL_COT, scalar2=None, op0=Alu.is_lt
            )
            pos = pp.tile([P, FW], f32, tag="pos")
            nc.vector.tensor_scalar(
                out=pos[:], in0=M[:], scalar1=0.0, scalar2=0.0, op0=Alu.is_gt,
                op1=Alu.add, accum_out=racc[:, 3:4],
            )

            # ---------------- segment one-hot + stats ------------------
            SR = mp.tile([P, W], i32, tag="SR")
            nc.sync.dma_start(
                out=SR[:], in_=t_seg[:].rearrange("(p r) w -> p r w", r=4)[:, 0, :]
            )
            segf = mp.tile([P, WF], f32, tag="segf")
            nc.vector.tensor_copy(
                out=segf[:], in_=SR[:].rearrange("p (j f) -> p j f", f=4)[:, :, 0]
            )
            # subsampled mask view: partition i holds rows 4i..4i+3; row 4i
            # occupies free offsets [0, 512); col 4j -> offset 4j
            Mi = M[:].rearrange("p (r w) -> p r w", r=4)[:, 0, :].rearrange(
                "p (j f) -> p j f", f=4
            )[:, :, 0]
            milt = mp.tile([P, WF], f32, tag="milt")
            nc.vector.tensor_scalar(
                out=milt[:], in0=Mi, scalar1=WALL_COT, scalar2=None, op0=Alu.is_lt
            )
            posi = mp.tile([P, WF], f32, tag="posi")
            nc.vector.scalar_tensor_tensor(
                out=posi[:], in0=Mi, scalar=0.0, in1=milt[:],
                op0=Alu.is_gt, op1=Alu.mult,
            )
            # one-hot [128, (j, g)]
            OH = bp.tile([P, WF * G], f32, tag="bigA")  # reuse E's slot
            OHv = OH[:].rearrange("p (j g) -> p j g", g=G)
            nc.vector.tensor_tensor(
                out=OHv,
                in0=segf[:, :, None].broadcast_to([P, WF, G]),
                in1=IO[:, None, :].broadcast_to([P, WF, G]),
                op=Alu.is_equal,
            )
            # packed rhs [128, (j, 3)] = (err, posi, 1)
            R = mp.tile([P, WF * 3], f32, tag="R")
            Rv = R[:].rearrange("p (j q) -> p j q", q=3)
            nc.scalar.copy(out=Rv[:, :, 0], in_=err_t[:])
            nc.scalar.copy(out=Rv[:, :, 1], in_=posi[:])
            nc.vector.memset(Rv[:, :, 2], 1.0)

            ps = qp.tile([G, 3], f32, tag="ps")
            for j in range(WF):
                nc.tensor.matmul(
                    ps[:], lhsT=OHv[:, j, :], rhs=Rv[:, j, :],
                    start=(j == 0), stop=(j == WF - 1),
                )
            segout = mp.tile([G, 3], f32, tag="segout")
            nc.vector.tensor_copy(out=segout[:], in_=ps[:])
            nc.sync.dma_start(out=t_segstats[:], in_=segout[:])

            # ---------------- recovery loss ----------------------------
            for c in range(C):
                o_t = ip.tile([P, FW], f32, tag="o")
                nc.sync.dma_start(
                    out=o_t[:], in_=t_out[c].rearrange("(p r) w -> p (r w)", p=P)
                )
                i_t = ip.tile([P, FW], f32, tag="i")
                nc.sync.dma_start(
                    out=i_t[:], in_=t_in[c].rearrange("(p r) w -> p (r w)", p=P)
                )
                tgt = sp.tile([P, FW], f32, tag="tgt")
                nc.vector.tensor_mul(tgt[:], i_t[:], m01[:])
                d_t = sp.tile([P, FW], f32, tag="d")
                nc.vector.tensor_sub(d_t[:], o_t[:], tgt[:])
                sq = sp.tile([P, FW], f32, tag="sq")
                nc.scalar.activation(out=sq[:], in_=d_t[:], func=Act.Square)
                # sq <- sq * pos, accumulate per-partition into racc[:, c]
                nc.vector.scalar_tensor_tensor(
                    out=sq[:], in0=sq[:], scalar=1.0, in1=pos[:],
                    op0=Alu.mult, op1=Alu.mult, accum_out=racc[:, c : c + 1],
                )
            nc.sync.dma_start(out=t_recov[:], in_=racc[:])

    nc.compile()
    return nc


def _get_nc():
    if "nc" not in _CACHE:
        _CACHE["nc"] = _build()
    return _CACHE["nc"]


def kernel(outputs, inputs, enc1, dec1, masks, segs, confidence=0, iteration=1,
           epoch=0, **_unused):
    from concourse.bass_utils import run_bass_kernel_spmd

    nc = _get_nc()
    iota = np.tile(np.arange(G, dtype=np.float32), (P, 1))
    in_maps = []
    for b in range(B):
        in_maps.append(
            {
                "outputs": np.ascontiguousarray(outputs[b]),
                "inputs": np.ascontiguousarray(inputs[b]),
                "enc1": np.ascontiguousarray(enc1[b]),
                "dec1": np.ascontiguousarray(dec1[b]),
                "masks": np.ascontiguousarray(masks[b, 0]),
                "segs": np.ascontiguousarray(segs[b, 0]),
                "iota": iota,
            }
        )
    res = run_bass_kernel_spmd(nc, in_maps, list(range(B)))

    seg_stats = np.stack([res.results[b]["seg_stats"] for b in range(B)])  # [B,G,3]
    recov = np.stack([res.results[b]["recov_stats"] for b in range(B)])  # [B,P,4]

    sum_err = seg_stats[:, :, 0] / np.float32(CF)
    pos_cnt = seg_stats[:, :, 1]
    counts = seg_stats[:, :, 2]

    valid = counts / np.float32(NPIX) >= np.float32(0.01)
    safe = np.maximum(counts, np.float32(1.0))
    mean_err = sum_err / safe
    flag = valid & (pos_cnt / safe > np.float32(0.01))
    sel = flag.astype(np.float32)
    flat_pos_mean = (mean_err * sel).sum(dtype=np.float64) / max(
        float(sel.sum(dtype=np.float64)), 1.0
    )

    wsum = recov[:, :, 0:3].sum(dtype=np.float64)
    cnt = recov[:, :, 3].sum(dtype=np.float64)
    loss_recov = wsum / max(cnt, 1.0)

    return np.float32(loss_recov + flat_pos_mean).reshape(())


# revision 7
# speedup vs baseline: 1.1481x; 1.1481x over previous
"""Trainium2 Bass kernel for nn_ConfidenceLossV2 (segment_reduce).

Strategy: pure data parallel over the batch dim (B=8 -> 8 NeuronCores,
one batch element per core). Each core computes:
  - seg_stats [64, 3]: per-segment (sum of channel-SUM sq err, pos count,
    pixel count) over the 128x128 subsampled grid
  - recov_stats [128, 4]: per-partition partial sums for the recovery loss
    (sum pos*d^2 for channels 0..2, sum pos)
Host gathers the tiny per-core partials and finishes the scalar math
(psum of (weighted sum, count) pairs), matching the reference exactly.
"""
import sys

if "/opt/trn_rl_repo" not in sys.path:
    sys.path.insert(0, "/opt/trn_rl_repo")

import numpy as np

B, C, H, W = 8, 3, 512, 512
CF, HF, WF = 64, 128, 128
G = 64
P = 128
WALL_COT = 0.5
NPIX = float(HF * WF)

_CACHE = {}


def _build():
    import concourse.bass as bass  # noqa: F401
    import concourse.tile as tile
    from concourse import bacc, mybir

    f32, i32 = mybir.dt.float32, mybir.dt.int32
    Alu = mybir.AluOpType
    Act = mybir.ActivationFunctionType

    nc = bacc.Bacc("TRN2", target_bir_lowering=False, debug=False, num_devices=B)

    t_out = nc.declare_dram_parameter("outputs", [C, H, W], f32, isOutput=False)
    t_in = nc.declare_dram_parameter("inputs", [C, H, W], f32, isOutput=False)
    t_enc = nc.declare_dram_parameter("enc1", [CF, HF, WF], f32, isOutput=False)
    t_dec = nc.declare_dram_parameter("dec1", [CF, HF, WF], f32, isOutput=False)
    t_mask = nc.declare_dram_parameter("masks", [H, W], f32, isOutput=False)
    t_seg = nc.declare_dram_parameter("segs", [H, W], i32, isOutput=False)
    t_iota = nc.declare_dram_parameter("iota", [P, G], f32, isOutput=False)
    t_segstats = nc.declare_dram_parameter("seg_stats", [3, G], f32, isOutput=True)
    t_recov = nc.declare_dram_parameter("recov_stats", [P, 4], f32, isOutput=True)

    FW = 2048  # free width of a [512,512] image tiled as [128, 2048]
    EF = CF * WF  # 8192 free for enc/dec as [128(h), 64(c)*128(w)]

    with tile.TileContext(nc) as tc:
        with (
            tc.tile_pool(name="persist", bufs=1) as pp,
            tc.tile_pool(name="img", bufs=2) as ip,
            tc.tile_pool(name="scr", bufs=1) as sp,
            tc.tile_pool(name="big", bufs=1) as bp,
            tc.tile_pool(name="small", bufs=1) as mp,
            tc.tile_pool(name="psum", bufs=1, space="PSUM") as qp,
        ):
            # ---------------- constants / accumulators ----------------
            IO = pp.tile([P, G], f32, tag="iota")
            nc.sync.dma_start(out=IO[:], in_=t_iota[:])
            racc = pp.tile([P, 4], f32, tag="racc")

            # ---------------- enc/dec reconstruction error -------------
            # layout [128 (h), 64 (c) * 128 (w)]
            E = bp.tile([P, EF], f32, tag="bigA")
            nc.sync.dma_start(
                out=E[:].rearrange("p (c w) -> p c w", c=CF),
                in_=t_enc[:].rearrange("c h w -> h c w"),
            )
            D = bp.tile([P, EF], f32, tag="bigB")
            nc.sync.dma_start(
                out=D[:].rearrange("p (c w) -> p c w", c=CF),
                in_=t_dec[:].rearrange("c h w -> h c w"),
            )
            DF = bp.tile([P, EF], f32, tag="bigC")
            nc.vector.tensor_sub(DF[:], E[:], D[:])
            # square in place (ACT engine)
            nc.scalar.activation(out=DF[:], in_=DF[:], func=Act.Square)
            # err[h, w] = sum_c sq[h, (c, w)] -- binary tree of adds on the
            # otherwise-idle gpsimd engine, in place in DF
            DFv = DF[:].rearrange("p (c w) -> p c w", c=CF)
            h = CF
            while h > 1:
                h //= 2
                nc.gpsimd.tensor_add(
                    DFv[:, 0:h, :], DFv[:, 0:h, :], DFv[:, h : 2 * h, :]
                )
            err_t = DFv[:, 0, :]  # [128, 128] view

            # ---------------- masks (full res, shared) -----------------
            M = pp.tile([P, FW], f32, tag="M")
            nc.sync.dma_start(
                out=M[:], in_=t_mask[:].rearrange("(p r) w -> p (r w)", p=P)
            )
            m01 = pp.tile([P, FW], f32, tag="m01")
            nc.vector.tensor_scalar(
                out=m01[:], in0=M[:], scalar1=WALL_COT, scalar2=None, op0=Alu.is_lt
            )
            pos = pp.tile([P, FW], f32, tag="pos")
            nc.vector.tensor_scalar(
                out=pos[:], in0=M[:], scalar1=0.0, scalar2=0.0, op0=Alu.is_gt,
                op1=Alu.add, accum_out=racc[:, 3:4],
            )

            # ---------------- segment one-hot + stats ------------------
            SR = mp.tile([P, W], i32, tag="SR")
            nc.sync.dma_start(
                out=SR[:], in_=t_seg[:].rearrange("(p r) w -> p r w", r=4)[:, 0, :]
            )
            segf = mp.tile([P, WF], f32, tag="segf")
            nc.vector.tensor_copy(
                out=segf[:], in_=SR[:].rearrange("p (j f) -> p j f", f=4)[:, :, 0]
            )
            # subsampled mask view: partition i holds rows 4i..4i+3; row 4i
            # occupies free offsets [0, 512); col 4j -> offset 4j
            Mi = M[:].rearrange("p (r w) -> p r w", r=4)[:, 0, :].rearrange(
                "p (j f) -> p j f", f=4
            )[:, :, 0]
            milt = mp.tile([P, WF], f32, tag="milt")
            nc.vector.tensor_scalar(
                out=milt[:], in0=Mi, scalar1=WALL_COT, scalar2=None, op0=Alu.is_lt
            )
            posi = mp.tile([P, WF], f32, tag="posi")
            nc.vector.scalar_tensor_tensor(
                out=posi[:], in0=Mi, scalar=0.0, in1=milt[:],
                op0=Alu.is_gt, op1=Alu.mult,
            )
            # one-hot [128, (j, g)]
            OH = bp.tile([P, WF * G], f32, tag="bigA")  # reuse E's slot
            OHv = OH[:].rearrange("p (j g) -> p j g", g=G)
            nc.vector.tensor_tensor(
                out=OHv,
                in0=segf[:, :, None].broadcast_to([P, WF, G]),
                in1=IO[:, None, :].broadcast_to([P, WF, G]),
                op=Alu.is_equal,
            )
            # packed rhs [128, (j, 3)] = (err, posi, 1)
            R = mp.tile([P, WF * 3], f32, tag="R")
            Rv = R[:].rearrange("p (j q) -> p j q", q=3)
            nc.scalar.copy(out=Rv[:, :, 0], in_=err_t)
            nc.scalar.copy(out=Rv[:, :, 1], in_=posi[:])
            nc.vector.memset(Rv[:, :, 2], 1.0)

            ps = qp.tile([3, G], f32, tag="ps")
            for j in range(WF):
                nc.tensor.matmul(
                    ps[:], lhsT=Rv[:, j, :], rhs=OHv[:, j, :],
                    start=(j == 0), stop=(j == WF - 1),
                )
            segout = mp.tile([3, G], f32, tag="segout")
            nc.vector.tensor_copy(out=segout[:], in_=ps[:])
            nc.sync.dma_start(out=t_segstats[:], in_=segout[:])

            # ---------------- recovery loss ----------------------------
            for c in range(C):
                o_t = ip.tile([P, FW], f32, tag="o")
                nc.sync.dma_start(
                    out=o_t[:], in_=t_out[c].rearrange("(p r) w -> p (r w)", p=P)
                )
                i_t = ip.tile([P, FW], f32, tag="i")
                nc.sync.dma_start(
                    out=i_t[:], in_=t_in[c].rearrange("(p r) w -> p (r w)", p=P)
                )
                tgt = sp.tile([P, FW], f32, tag="tgt")
                nc.vector.tensor_mul(tgt[:], i_t[:], m01[:])
                d_t = sp.tile([P, FW], f32, tag="d")
                nc.vector.tensor_sub(d_t[:], o_t[:], tgt[:])
                sq = sp.tile([P, FW], f32, tag="sq")
                nc.scalar.activation(out=sq[:], in_=d_t[:], func=Act.Square)
                # sq <- sq * pos, accumulate per-partition into racc[:, c]
                nc.vector.scalar_tensor_tensor(
                    out=sq[:], in0=sq[:], scalar=1.0, in1=pos[:],
                    op0=Alu.mult, op1=Alu.mult, accum_out=racc[:, c : c + 1],
                )
            nc.sync.dma_start(out=t_recov[:], in_=racc[:])

    nc.compile()
    return nc


def _get_nc():
    if "nc" not in _CACHE:
        _CACHE["nc"] = _build()
    return _CACHE["nc"]


def kernel(outputs, inputs, enc1, dec1, masks, segs, confidence=0, iteration=1,
           epoch=0, **_unused):
    from concourse.bass_utils import run_bass_kernel_spmd

    nc = _get_nc()
    iota = np.tile(np.arange(G, dtype=np.float32), (P, 1))
    in_maps = []
    for b in range(B):
        in_maps.append(
            {
                "outputs": np.ascontiguousarray(outputs[b]),
                "inputs": np.ascontiguousarray(inputs[b]),
                "enc1": np.ascontiguousarray(enc1[b]),
                "dec1": np.ascontiguousarray(dec1[b]),
                "masks": np.ascontiguousarray(masks[b, 0]),
                "segs": np.ascontiguousarray(segs[b, 0]),
                "iota": iota,
            }
        )
    res = run_bass_kernel_spmd(nc, in_maps, list(range(B)))

    seg_stats = np.stack([res.results[b]["seg_stats"] for b in range(B)])  # [B,3,G]
    recov = np.stack([res.results[b]["recov_stats"] for b in range(B)])  # [B,P,4]

    sum_err = seg_stats[:, 0, :] / np.float32(CF)
    pos_cnt = seg_stats[:, 1, :]
    counts = seg_stats[:, 2, :]

    valid = counts / np.float32(NPIX) >= np.float32(0.01)
    safe = np.maximum(counts, np.float32(1.0))
    mean_err = sum_err / safe
    flag = valid & (pos_cnt / safe > np.float32(0.01))
    sel = flag.astype(np.float32)
    flat_pos_mean = (mean_err * sel).sum(dtype=np.float64) / max(
        float(sel.sum(dtype=np.float64)), 1.0
    )

    wsum = recov[:, :, 0:3].sum(dtype=np.float64)
    cnt = recov[:, :, 3].sum(dtype=np.float64)
    loss_recov = wsum / max(cnt, 1.0)

    return np.float32(loss_recov + flat_pos_mean).reshape(())


# revision 8
# speedup vs baseline: 1.3728x; 1.1957x over previous
"""Trainium2 Bass kernel for nn_ConfidenceLossV2 (segment_reduce).

Pure data parallel over the batch dim (B=8 -> 8 NeuronCores, one batch
element per core). Per core outputs:
  - seg_stats [12, 256]: packed PSUM blocks; diagonal 3x64 blocks hold
    (sum of channel-SUM sq err, pos count, pixel count) per segment
  - recov_stats [128, 4]: per-partition partial sums for the recovery
    loss (sum pos*d^2 for channels 0..2, sum pos)
Host gathers the tiny per-core partials and finishes the scalar math.
"""
import sys

if "/opt/trn_rl_repo" not in sys.path:
    sys.path.insert(0, "/opt/trn_rl_repo")

import numpy as np

B, C, H, W = 8, 3, 512, 512
CF, HF, WF = 64, 128, 128
G = 64
P = 128
WALL_COT = 0.5
NPIX = float(HF * WF)
NCH = 16            # channels per enc/dec chunk
NCK = CF // NCH     # 4 chunks
MMJ = 4             # j-columns packed per matmul

_CACHE = {}


def _build():
    import concourse.bass as bass  # noqa: F401
    import concourse.tile as tile
    from concourse import bacc, mybir

    f32, i32 = mybir.dt.float32, mybir.dt.int32
    Alu = mybir.AluOpType
    Act = mybir.ActivationFunctionType

    nc = bacc.Bacc("TRN2", target_bir_lowering=False, debug=False, num_devices=B)

    t_out = nc.declare_dram_parameter("outputs", [C, H, W], f32, isOutput=False)
    t_in = nc.declare_dram_parameter("inputs", [C, H, W], f32, isOutput=False)
    t_enc = nc.declare_dram_parameter("enc1", [CF, HF, WF], f32, isOutput=False)
    t_dec = nc.declare_dram_parameter("dec1", [CF, HF, WF], f32, isOutput=False)
    t_mask = nc.declare_dram_parameter("masks", [H, W], f32, isOutput=False)
    t_seg = nc.declare_dram_parameter("segs", [H, W], i32, isOutput=False)
    t_iota = nc.declare_dram_parameter("iota", [P, G], f32, isOutput=False)
    t_segstats = nc.declare_dram_parameter(
        "seg_stats", [3 * MMJ, G * MMJ], f32, isOutput=True
    )
    t_recov = nc.declare_dram_parameter("recov_stats", [P, 4], f32, isOutput=True)

    FW = 2048           # free width of a [512,512] image tiled as [128, 2048]
    CW = NCH * WF       # 2048 free per enc/dec chunk

    with tile.TileContext(nc) as tc:
        with (
            tc.tile_pool(name="persist", bufs=1) as pp,
            tc.tile_pool(name="img", bufs=2) as ip,
            tc.tile_pool(name="scr", bufs=2) as sp,
            tc.tile_pool(name="chunk", bufs=2) as cp,
            tc.tile_pool(name="big", bufs=1) as bp,
            tc.tile_pool(name="small", bufs=1) as mp,
            tc.tile_pool(name="psum", bufs=1, space="PSUM") as qp,
        ):
            # ---- constants / accumulators / small DMAs ----------------
            IO = pp.tile([P, G], f32, tag="iota")
            nc.sync.dma_start(out=IO[:], in_=t_iota[:])
            racc = pp.tile([P, 4], f32, tag="racc")

            SR = mp.tile([P, W], i32, tag="SR")
            nc.sync.dma_start(
                out=SR[:], in_=t_seg[:].rearrange("(p r) w -> p r w", r=4)[:, 0, :]
            )
            M = pp.tile([P, FW], f32, tag="M")
            nc.scalar.dma_start(
                out=M[:], in_=t_mask[:].rearrange("(p r) w -> p (r w)", p=P)
            )

            # ---- one-hot early (DVE) so PE can start asap -------------
            segf = mp.tile([P, WF], f32, tag="segf")
            nc.vector.tensor_copy(
                out=segf[:], in_=SR[:].rearrange("p (j f) -> p j f", f=4)[:, :, 0]
            )
            OH = bp.tile([P, WF * G], f32, tag="bigOH")
            OHv = OH[:].rearrange("p (j g) -> p j g", g=G)
            nc.vector.tensor_tensor(
                out=OHv,
                in0=segf[:, :, None].broadcast_to([P, WF, G]),
                in1=IO[:, None, :].broadcast_to([P, WF, G]),
                op=Alu.is_equal,
            )

            # ---- masks ------------------------------------------------
            m01 = pp.tile([P, FW], f32, tag="m01")
            nc.vector.tensor_scalar(
                out=m01[:], in0=M[:], scalar1=WALL_COT, scalar2=None, op0=Alu.is_lt
            )
            pos = pp.tile([P, FW], f32, tag="pos")
            nc.vector.tensor_scalar(
                out=pos[:], in0=M[:], scalar1=0.0, scalar2=0.0, op0=Alu.is_gt,
                op1=Alu.add, accum_out=racc[:, 3:4],
            )
            Bm = pp.tile([P, FW], f32, tag="Bm")
            nc.vector.tensor_mul(Bm[:], m01[:], pos[:])

            # subsampled-mask segment masks
            Mi = M[:].rearrange("p (r w) -> p r w", r=4)[:, 0, :].rearrange(
                "p (j f) -> p j f", f=4
            )[:, :, 0]
            milt = mp.tile([P, WF], f32, tag="milt")
            nc.vector.tensor_scalar(
                out=milt[:], in0=Mi, scalar1=WALL_COT, scalar2=None, op0=Alu.is_lt
            )
            posi = mp.tile([P, WF], f32, tag="posi")
            nc.vector.scalar_tensor_tensor(
                out=posi[:], in0=Mi, scalar=0.0, in1=milt[:],
                op0=Alu.is_gt, op1=Alu.mult,
            )

            # ---- enc/dec chunks: DMA(sync) -> sub(DVE) -> sq(ACT) -----
            # -> per-chunk c-reduce(DVE) into PK columns
            PK = mp.tile([P, NCK * WF], f32, tag="PK")
            PKv = PK[:].rearrange("p (k w) -> p k w", k=NCK)
            enc_v = t_enc[:].rearrange("c h w -> h c w")
            dec_v = t_dec[:].rearrange("c h w -> h c w")
            for k in range(NCK):
                Ek = cp.tile([P, CW], f32, tag="E")
                nc.sync.dma_start(
                    out=Ek[:].rearrange("p (c w) -> p c w", c=NCH),
                    in_=enc_v[:, k * NCH : (k + 1) * NCH, :],
                )
                Dk = cp.tile([P, CW], f32, tag="D")
                nc.sync.dma_start(
                    out=Dk[:].rearrange("p (c w) -> p c w", c=NCH),
                    in_=dec_v[:, k * NCH : (k + 1) * NCH, :],
                )
                nc.vector.tensor_sub(Ek[:], Ek[:], Dk[:])
                nc.scalar.activation(out=Ek[:], in_=Ek[:], func=Act.Square)
                nc.vector.reduce_sum(
                    out=PKv[:, k, :],
                    in_=Ek[:].rearrange("p (c w) -> p w c", c=NCH),
                    axis=mybir.AxisListType.X,
                )
            err_t = mp.tile([P, WF], f32, tag="err")
            nc.vector.reduce_sum(
                out=err_t[:],
                in_=PK[:].rearrange("p (k w) -> p w k", k=NCK),
                axis=mybir.AxisListType.X,
            )

            # ---- packed rhs + matmuls ---------------------------------
            R = mp.tile([P, WF * 3], f32, tag="R")
            Rv = R[:].rearrange("p (j q) -> p j q", q=3)
            nc.scalar.copy(out=Rv[:, :, 0], in_=err_t[:])
            nc.scalar.copy(out=Rv[:, :, 1], in_=posi[:])
            nc.vector.memset(Rv[:, :, 2], 1.0)

            ps = qp.tile([3 * MMJ, G * MMJ], f32, tag="ps")
            NT = WF // MMJ
            for t in range(NT):
                nc.tensor.matmul(
                    ps[:],
                    lhsT=Rv[:, t * MMJ : (t + 1) * MMJ, :],
                    rhs=OHv[:, t * MMJ : (t + 1) * MMJ, :],
                    start=(t == 0), stop=(t == NT - 1),
                )
            segout = mp.tile([3 * MMJ, G * MMJ], f32, tag="segout")
            nc.vector.tensor_copy(out=segout[:], in_=ps[:])
            nc.scalar.dma_start(out=t_segstats[:], in_=segout[:])

            # ---- recovery: d' = o*pos - i*(m01*pos); ACT Square+accum -
            for c in range(C):
                o_t = ip.tile([P, FW], f32, tag="o")
                nc.scalar.dma_start(
                    out=o_t[:], in_=t_out[c].rearrange("(p r) w -> p (r w)", p=P)
                )
                i_t = ip.tile([P, FW], f32, tag="i")
                nc.scalar.dma_start(
                    out=i_t[:], in_=t_in[c].rearrange("(p r) w -> p (r w)", p=P)
                )
                op_t = sp.tile([P, FW], f32, tag="op")
                nc.vector.tensor_mul(op_t[:], o_t[:], pos[:])
                tp_t = sp.tile([P, FW], f32, tag="tp")
                nc.vector.tensor_mul(tp_t[:], i_t[:], Bm[:])
                nc.vector.tensor_sub(op_t[:], op_t[:], tp_t[:])
                sq_t = sp.tile([P, FW], f32, tag="sq")
                nc.scalar.activation(
                    out=sq_t[:], in_=op_t[:], func=Act.Square,
                    accum_out=racc[:, c : c + 1],
                )
            nc.scalar.dma_start(out=t_recov[:], in_=racc[:])

    nc.compile()
    return nc


def _get_nc():
    if "nc" not in _CACHE:
        _CACHE["nc"] = _build()
    return _CACHE["nc"]


def _in_maps(outputs, inputs, enc1, dec1, masks, segs):
    iota = np.tile(np.arange(G, dtype=np.float32), (P, 1))
    maps = []
    for b in range(B):
        maps.append(
            {
                "outputs": np.ascontiguousarray(outputs[b]),
                "inputs": np.ascontiguousarray(inputs[b]),
                "enc1": np.ascontiguousarray(enc1[b]),
                "dec1": np.ascontiguousarray(dec1[b]),
                "masks": np.ascontiguousarray(masks[b, 0]),
                "segs": np.ascontiguousarray(segs[b, 0]),
                "iota": iota,
            }
        )
    return maps


def kernel(outputs, inputs, enc1, dec1, masks, segs, confidence=0, iteration=1,
           epoch=0, **_unused):
    from concourse.bass_utils import run_bass_kernel_spmd

    nc = _get_nc()
    res = run_bass_kernel_spmd(
        nc, _in_maps(outputs, inputs, enc1, dec1, masks, segs), list(range(B))
    )

    raw = np.stack([res.results[b]["seg_stats"] for b in range(B)])  # [B,12,256]
    recov = np.stack([res.results[b]["recov_stats"] for b in range(B)])  # [B,P,4]

    # sum the MMJ diagonal blocks -> [B, 3, G]
    seg_stats = np.zeros((B, 3, G), np.float32)
    for u in range(MMJ):
        seg_stats += raw[:, 3 * u : 3 * u + 3, G * u : G * u + G]

    sum_err = seg_stats[:, 0, :] / np.float32(CF)
    pos_cnt = seg_stats[:, 1, :]
    counts = seg_stats[:, 2, :]

    valid = counts / np.float32(NPIX) >= np.float32(0.01)
    safe = np.maximum(counts, np.float32(1.0))
    mean_err = sum_err / safe
    flag = valid & (pos_cnt / safe > np.float32(0.01))
    sel = flag.astype(np.float32)
    flat_pos_mean = (mean_err * sel).sum(dtype=np.float64) / max(
        float(sel.sum(dtype=np.float64)), 1.0
    )

    wsum = recov[:, :, 0:3].sum(dtype=np.float64)
    cnt = recov[:, :, 3].sum(dtype=np.float64)
    loss_recov = wsum / max(cnt, 1.0)

    return np.float32(loss_recov + flat_pos_mean).reshape(())
